# Initial kernel scaffold
#
"""Trainium2 Bass kernel for the ClusterLoss problem.

Loss = mean-entropy(softmax over K of [T, M, K] logits)            (L1)
       - mean-entropy(softmax over K of batch-mean logits [M, K])  (L2)

T=4096, M=64, K=256 hardcoded. Data-parallel over T across 8 cores.

Per core (shard = [512*64, 256] rows, viewed as 256 tiles of [128, 256]):
  - DMA 1 MiB mega-tiles (8 row-tiles) into SBUF.
  - ACT:  e = exp(x) per tile slice, accum_out -> Z[row] column buffer.
  - DVE:  w = x*e (in-place over e), accum_out -> S[row] column buffer
          (fused tensor_tensor_reduce).
  - PE:   0/1-pattern matmul accumulates per-block sums over T into PSUM
          (for L2's batch-mean logits).
  - tail: H_sum/partition = sum_tiles ln(Z) - sum_tiles S*(1/Z),
          computed batched on the [128, 256] stat buffers.
Outputs per core: ent [128,1] (partition-sums of per-row entropies) and
bsum [64,256] (partial sum over this core's T rows). Host reduces those
tiny tensors into the final scalar.

No max-subtraction in the softmax: inputs are standard-normal, |x| < ~6,
so exp(x) is comfortably inside fp32 range; H = ln(Z) - S/Z is
analytically identical to the reference's log_softmax entropy.
"""

import numpy as np

import concourse.bacc as bacc
import concourse.bass as bass
import concourse.tile as tile
from concourse import mybir
from concourse.bass_utils import run_bass_kernel_spmd

T, M, K = 4096, 64, 256
NCORES = 8
TSH = T // NCORES            # 512 t-rows per core
ROWS = TSH * M               # 32768 (t, m) rows per core
P = 128                      # SBUF partitions per tile
NTILES = ROWS // P           # 256 tiles of [128, 256] per core
MEGA = 8                     # row-tiles per DMA (1 MiB transfers)
NMEGA = NTILES // MEGA       # 32
PAIR = 2                     # row-tiles per PE matmul (moving free dim 512)
BSPLIT = 3                   # per mega-tile: last BSPLIT tiles use batched
                             # exp + DVE Z-reduce (ACT/DVE load balance)
BSPLIT_ALT = True            # odd mega-tiles use BSPLIT+1 (avg +0.5)

FP32 = mybir.dt.float32
FP32R = mybir.dt.float32r


def _build_nc(use_pe=True, use_act=True, use_dve=True, repeat=1):
    from contextlib import nullcontext

    nc = bacc.Bacc("TRN2", target_bir_lowering=False, debug=False)

    x_d = nc.dram_tensor("x", [NTILES, P, K], FP32R, kind="ExternalInput")
    w_d = nc.dram_tensor("wpat", [P, M], FP32R, kind="ExternalInput")
    ent_d = nc.dram_tensor("ent", [P, 1], FP32, kind="ExternalOutput")
    bsum_d = nc.dram_tensor("bsum", [M, K], FP32, kind="ExternalOutput")

    x = x_d.ap()

    with tile.TileContext(nc) as tc:
        with (
            tc.tile_pool(name="xin", bufs=4) as xpool,
            tc.tile_pool(name="exp", bufs=4) as epool,
            tc.tile_pool(name="stats", bufs=1) as stats,
            tc.tile_pool(name="small", bufs=1) as small,
            tc.tile_pool(name="psum", bufs=1, space="PSUM") as psum,
            tc.For_i(0, repeat, 1) if repeat > 1 else nullcontext(),
        ):
            wp = small.tile([P, M], FP32R)
            nc.sync.dma_start(out=wp, in_=w_d.ap())

            # Stats columns split by mode so each buffer has one writer
            # engine (ACT writes *_a accums; DVE writes everything else).
            nb_per_mega = BSPLIT if (use_act and use_dve) else (0 if use_act else MEGA)
            n_b = NMEGA * nb_per_mega              # tiles in batched mode
            if nb_per_mega == BSPLIT and BSPLIT_ALT:
                n_b += NMEGA // 2
            n_a = NTILES - n_b
            zbuf_a = stats.tile([P, max(n_a, 1)], FP32)
            sbuf_a = stats.tile([P, max(n_a, 1)], FP32)
            zbuf_b = stats.tile([P, max(n_b, 1)], FP32)
            sbuf_b = stats.tile([P, max(n_b, 1)], FP32)
            bs_ps = psum.tile([M, PAIR, K], FP32)  # block sums (2 halves)

            ia = ib = 0
            for mg in range(NMEGA):
                xtr = xpool.tile([P, MEGA, K], FP32R)
                nc.sync.dma_start(
                    out=xtr,
                    in_=x[mg * MEGA:(mg + 1) * MEGA].rearrange("c p k -> p c k"),
                )
                xt = xtr.bitcast(FP32)
                et = epool.tile([P, MEGA, K], FP32)
                nb_here = nb_per_mega
                if nb_per_mega == BSPLIT and BSPLIT_ALT and (mg % 2):
                    nb_here = BSPLIT + 1
                na_here = MEGA - nb_here
                # mode A tiles (slices 0..na_here): fused exp+Z on ACT
                for j in range(na_here):
                    if use_act:
                        nc.scalar.activation(
                            out=et[:, j, :],
                            in_=xt[:, j, :],
                            func=mybir.ActivationFunctionType.Exp,
                            accum_out=zbuf_a[:, ia:ia + 1],
                        )
                    if use_dve:
                        nc.vector.affine_mul_reduce(
                            out=et[:, j, :],
                            accum_out=sbuf_a[:, ia:ia + 1],
                            in0=xt[:, j, :],
                            in1=et[:, j, :],
                            scale=1.0,
                            bias=0.0,
                        )
                    ia += 1
                # mode B tiles (remaining slices): one batched exp, Z on DVE
                if nb_here:
                    if use_act:
                        nc.scalar.activation(
                            out=et[:, na_here:, :], in_=xt[:, na_here:, :],
                            func=mybir.ActivationFunctionType.Exp,
                        )
                    if use_dve:
                        # all nb_here per-row Z sums in one 3D reduce
                        nc.vector.tensor_reduce(
                            out=zbuf_b[:, ib:ib + nb_here],
                            in_=et[:, na_here:, :],
                            axis=mybir.AxisListType.X,
                            op=mybir.AluOpType.add,
                        )
                    for j in range(na_here, MEGA):
                        if use_dve:
                            nc.vector.affine_mul_reduce(
                                out=et[:, j, :],
                                accum_out=sbuf_b[:, ib:ib + 1],
                                in0=xt[:, j, :],
                                in1=et[:, j, :],
                                scale=1.0,
                                bias=0.0,
                            )
                        ib += 1
                if use_pe:
                    for j in range(MEGA // PAIR):
                        g = mg * (MEGA // PAIR) + j
                        nc.tensor.matmul(
                            bs_ps,
                            wp,
                            xtr[:, j * PAIR:(j + 1) * PAIR, :],
                            start=(g == 0),
                            stop=(g == NMEGA * (MEGA // PAIR) - 1),
                        )

            # ---- tail: batched entropy math over the stat buffers ----
            parts = []
            for idx, (zb, sb, n) in enumerate(
                ((zbuf_a, sbuf_a, n_a), (zbuf_b, sbuf_b, n_b))
            ):
                if n == 0:
                    continue
                if not (use_act and use_dve):
                    nc.vector.memset(zb, 1.0)
                    nc.vector.memset(sb, 0.0)
                logz = stats.tile([P, n], FP32, tag=f"logz{idx}")
                lsum = small.tile([P, 1], FP32, tag=f"lsum{idx}")
                nc.scalar.activation(
                    out=logz, in_=zb,
                    func=mybir.ActivationFunctionType.Ln,
                    accum_out=lsum,
                )
                rz = stats.tile([P, n], FP32, tag=f"rz{idx}")
                nc.vector.reciprocal(out=rz, in_=zb)
                szsum = small.tile([P, 1], FP32, tag=f"szsum{idx}")
                nc.vector.affine_mul_reduce(
                    out=rz, accum_out=szsum,
                    in0=sb, in1=rz,
                    scale=1.0, bias=0.0,
                )
                part = small.tile([P, 1], FP32, tag=f"part{idx}")
                nc.vector.tensor_sub(part, lsum, szsum)
                parts.append(part)
            ent_sb = small.tile([P, 1], FP32)
            if len(parts) == 2:
                nc.vector.tensor_add(ent_sb, parts[0], parts[1])
            else:
                nc.vector.tensor_copy(out=ent_sb, in_=parts[0])
            nc.sync.dma_start(out=ent_d.ap(), in_=ent_sb)

            bsum_sb = small.tile([M, K], FP32)
            if use_pe:
                nc.scalar.copy(bsum_sb, bs_ps[:, 0, :])
                nc.vector.tensor_add(bsum_sb, bsum_sb, bs_ps[:, 1, :])
            else:
                nc.vector.memset(bsum_sb, 0.0)
            nc.sync.dma_start(out=bsum_d.ap(), in_=bsum_sb)

    nc.compile()
    return nc


_NC_CACHE = []


def _get_nc():
    if not _NC_CACHE:
        _NC_CACHE.append(_build_nc())
    return _NC_CACHE[0]


def _wpat():
    wp = np.zeros((P, M), np.float32)
    wp[np.arange(P), np.arange(P) % M] = 1.0
    return wp


def kernel(block_feats, **kw):
    assert int(kw.get("M", M)) == M
    xf = np.ascontiguousarray(np.asarray(block_feats, dtype=np.float32))
    assert xf.shape == (T, M * K)
    shards = xf.reshape(NCORES, NTILES, P, K)

    nc = _get_nc()
    wp = _wpat()
    in_maps = [{"x": shards[i], "wpat": wp} for i in range(NCORES)]
    res = run_bass_kernel_spmd(nc, in_maps, core_ids=list(range(NCORES))).results

    ent_total = sum(float(r["ent"].sum(dtype=np.float64)) for r in res)
    L1 = ent_total / (T * M)

    bs = np.zeros((M, K), np.float64)
    for r in res:
        bs += r["bsum"]
    bm = bs / T
    z = bm - bm.max(axis=-1, keepdims=True)
    e = np.exp(z)
    Z = e.sum(axis=-1, keepdims=True)
    logp = z - np.log(Z)
    H = -(np.exp(logp) * logp).sum(axis=-1)
    L2 = -H.mean()

    return np.asarray(L1 + L2, dtype=np.float32)



# revision 1
# speedup vs baseline: 1.8940x; 1.8940x over previous
"""Trainium2 Bass kernel for the ClusterLoss problem.

Loss = mean-entropy(softmax over K of [T, M, K] logits)            (L1)
       - mean-entropy(softmax over K of batch-mean logits [M, K])  (L2)

T=4096, M=64, K=256 hardcoded. Data-parallel over T across 8 cores.

Per core (shard = [512*64, 256] rows, viewed as 256 tiles of [128, 256]):
  - DMA 1 MiB mega-tiles (8 row-tiles) into SBUF.
  - ACT:  e = exp(x) per tile slice, accum_out -> Z[row] column buffer.
  - DVE:  w = x*e (in-place over e), accum_out -> S[row] column buffer
          (fused tensor_tensor_reduce).
  - PE:   0/1-pattern matmul accumulates per-block sums over T into PSUM
          (for L2's batch-mean logits).
  - tail: H_sum/partition = sum_tiles ln(Z) - sum_tiles S*(1/Z),
          computed batched on the [128, 256] stat buffers.
Outputs per core: ent [128,1] (partition-sums of per-row entropies) and
bsum [64,256] (partial sum over this core's T rows). Host reduces those
tiny tensors into the final scalar.

No max-subtraction in the softmax: inputs are standard-normal, |x| < ~6,
so exp(x) is comfortably inside fp32 range; H = ln(Z) - S/Z is
analytically identical to the reference's log_softmax entropy.
"""

import numpy as np

import concourse.bacc as bacc
import concourse.bass as bass
import concourse.tile as tile
from concourse import mybir
from concourse.bass_utils import run_bass_kernel_spmd

T, M, K = 4096, 64, 256
NCORES = 8
TSH = T // NCORES            # 512 t-rows per core
ROWS = TSH * M               # 32768 (t, m) rows per core
P = 128                      # SBUF partitions per tile
NTILES = ROWS // P           # 256 tiles of [128, 256] per core
MEGA = 8                     # row-tiles per DMA (1 MiB transfers)
NMEGA = NTILES // MEGA       # 32
PAIR = 2                     # row-tiles per PE matmul (moving free dim 512)
BSPLIT = 3                   # per mega-tile: last BSPLIT tiles use batched
                             # exp + DVE Z-reduce (ACT/DVE load balance)
BSPLIT_ALT = True            # odd mega-tiles use BSPLIT+1 (avg +0.5)

FP32 = mybir.dt.float32
FP32R = mybir.dt.float32r


def _build_nc(use_pe=True, use_act=True, use_dve=True, repeat=1):
    from contextlib import nullcontext

    nc = bacc.Bacc("TRN2", target_bir_lowering=False, debug=False)

    x_d = nc.dram_tensor("x", [NTILES, P, K], FP32R, kind="ExternalInput")
    w_d = nc.dram_tensor("wpat", [P, M], FP32R, kind="ExternalInput")
    ent_d = nc.dram_tensor("ent", [P, 1], FP32, kind="ExternalOutput")
    bsum_d = nc.dram_tensor("bsum", [M, K], FP32, kind="ExternalOutput")

    x = x_d.ap()

    with tile.TileContext(nc) as tc:
        with (
            tc.tile_pool(name="xin", bufs=4) as xpool,
            tc.tile_pool(name="exp", bufs=4) as epool,
            tc.tile_pool(name="stats", bufs=1) as stats,
            tc.tile_pool(name="small", bufs=1) as small,
            tc.tile_pool(name="psum", bufs=1, space="PSUM") as psum,
            tc.For_i(0, repeat, 1) if repeat > 1 else nullcontext(),
        ):
            wp = small.tile([P, M], FP32R)
            nc.sync.dma_start(out=wp, in_=w_d.ap())

            # Stats columns split by mode so each buffer has one writer
            # engine (ACT writes *_a accums; DVE writes everything else).
            nb_per_mega = BSPLIT if (use_act and use_dve) else (0 if use_act else MEGA)
            n_b = NMEGA * nb_per_mega              # tiles in batched mode
            if nb_per_mega == BSPLIT and BSPLIT_ALT:
                n_b += NMEGA // 2
            n_a = NTILES - n_b
            zbuf_a = stats.tile([P, max(n_a, 1)], FP32)
            sbuf_a = stats.tile([P, max(n_a, 1)], FP32)
            zbuf_b = stats.tile([P, max(n_b, 1)], FP32)
            sbuf_b = stats.tile([P, max(n_b, 1)], FP32)
            bs_ps = psum.tile([M, PAIR, K], FP32)  # block sums (2 halves)

            ia = ib = 0
            for mg in range(NMEGA):
                xtr = xpool.tile([P, MEGA, K], FP32R)
                nc.sync.dma_start(
                    out=xtr,
                    in_=x[mg * MEGA:(mg + 1) * MEGA].rearrange("c p k -> p c k"),
                )
                xt = xtr.bitcast(FP32)
                et = epool.tile([P, MEGA, K], FP32)
                nb_here = nb_per_mega
                if nb_per_mega == BSPLIT and BSPLIT_ALT and (mg % 2):
                    nb_here = BSPLIT + 1
                na_here = MEGA - nb_here
                # mode A tiles (slices 0..na_here): fused exp+Z on ACT
                for j in range(na_here):
                    if use_act:
                        nc.scalar.activation(
                            out=et[:, j, :],
                            in_=xt[:, j, :],
                            func=mybir.ActivationFunctionType.Exp,
                            accum_out=zbuf_a[:, ia:ia + 1],
                        )
                    if use_dve:
                        nc.vector.affine_mul_reduce(
                            out=et[:, j, :],
                            accum_out=sbuf_a[:, ia:ia + 1],
                            in0=xt[:, j, :],
                            in1=et[:, j, :],
                            scale=1.0,
                            bias=0.0,
                        )
                    ia += 1
                # mode B tiles (remaining slices): one batched exp, Z on DVE
                if nb_here:
                    if use_act:
                        nc.scalar.activation(
                            out=et[:, na_here:, :], in_=xt[:, na_here:, :],
                            func=mybir.ActivationFunctionType.Exp,
                        )
                    if use_dve:
                        # all nb_here per-row Z sums in one 3D reduce
                        nc.vector.tensor_reduce(
                            out=zbuf_b[:, ib:ib + nb_here],
                            in_=et[:, na_here:, :],
                            axis=mybir.AxisListType.X,
                            op=mybir.AluOpType.add,
                        )
                    for j in range(na_here, MEGA):
                        if use_dve:
                            nc.vector.affine_mul_reduce(
                                out=et[:, j, :],
                                accum_out=sbuf_b[:, ib:ib + 1],
                                in0=xt[:, j, :],
                                in1=et[:, j, :],
                                scale=1.0,
                                bias=0.0,
                            )
                        ib += 1
                if use_pe:
                    for j in range(MEGA // PAIR):
                        g = mg * (MEGA // PAIR) + j
                        nc.tensor.matmul(
                            bs_ps,
                            wp,
                            xtr[:, j * PAIR:(j + 1) * PAIR, :],
                            start=(g == 0),
                            stop=(g == NMEGA * (MEGA // PAIR) - 1),
                        )

            # ---- tail: batched entropy math over the stat buffers ----
            parts = []
            for idx, (zb, sb, n) in enumerate(
                ((zbuf_a, sbuf_a, n_a), (zbuf_b, sbuf_b, n_b))
            ):
                if n == 0:
                    continue
                if not (use_act and use_dve):
                    nc.vector.memset(zb, 1.0)
                    nc.vector.memset(sb, 0.0)
                logz = stats.tile([P, n], FP32, tag=f"logz{idx}")
                lsum = small.tile([P, 1], FP32, tag=f"lsum{idx}")
                nc.scalar.activation(
                    out=logz, in_=zb,
                    func=mybir.ActivationFunctionType.Ln,
                    accum_out=lsum,
                )
                rz = stats.tile([P, n], FP32, tag=f"rz{idx}")
                nc.vector.reciprocal(out=rz, in_=zb)
                szsum = small.tile([P, 1], FP32, tag=f"szsum{idx}")
                nc.vector.affine_mul_reduce(
                    out=rz, accum_out=szsum,
                    in0=sb, in1=rz,
                    scale=1.0, bias=0.0,
                )
                part = small.tile([P, 1], FP32, tag=f"part{idx}")
                nc.vector.tensor_sub(part, lsum, szsum)
                parts.append(part)
            ent_sb = small.tile([P, 1], FP32)
            if len(parts) == 2:
                nc.vector.tensor_add(ent_sb, parts[0], parts[1])
            else:
                nc.vector.tensor_copy(out=ent_sb, in_=parts[0])
            nc.sync.dma_start(out=ent_d.ap(), in_=ent_sb)

            bsum_sb = small.tile([M, K], FP32)
            if use_pe:
                nc.scalar.copy(bsum_sb, bs_ps[:, 0, :])
                nc.vector.tensor_add(bsum_sb, bsum_sb, bs_ps[:, 1, :])
            else:
                nc.vector.memset(bsum_sb, 0.0)
            nc.sync.dma_start(out=bsum_d.ap(), in_=bsum_sb)

    nc.compile()
    return nc


_NC_CACHE = []


def _get_nc():
    if not _NC_CACHE:
        _NC_CACHE.append(_build_nc())
    return _NC_CACHE[0]


def _wpat():
    wp = np.zeros((P, M), np.float32)
    wp[np.arange(P), np.arange(P) % M] = 1.0
    return wp


def kernel(block_feats, **kw):
    assert int(kw.get("M", M)) == M
    xf = np.ascontiguousarray(np.asarray(block_feats, dtype=np.float32))
    assert xf.shape == (T, M * K)
    shards = xf.reshape(NCORES, NTILES, P, K)

    nc = _get_nc()
    wp = _wpat()
    in_maps = [{"x": shards[i], "wpat": wp} for i in range(NCORES)]
    res = run_bass_kernel_spmd(nc, in_maps, core_ids=list(range(NCORES))).results

    ent_total = sum(float(r["ent"].sum(dtype=np.float64)) for r in res)
    L1 = ent_total / (T * M)

    bs = np.zeros((M, K), np.float64)
    for r in res:
        bs += r["bsum"]
    bm = bs / T
    z = bm - bm.max(axis=-1, keepdims=True)
    e = np.exp(z)
    Z = e.sum(axis=-1, keepdims=True)
    logp = z - np.log(Z)
    H = -(np.exp(logp) * logp).sum(axis=-1)
    L2 = -H.mean()

    return np.asarray(L1 + L2, dtype=np.float32)

